# revision 5
# baseline (speedup 1.0000x reference)
"""Trainium2 Bass kernel for NoTPAttention (dense transformer block:
fused QKV projection -> multi-head attention -> output projection).

Sharding (8 NeuronCores): core c handles batch b = c // 4 and the 4 heads
g = 4*(c % 4) .. 4*(c % 4)+3 (head-parallel tensor parallelism).  Each core
computes its heads' partial out-projection [S, H] in bf16; the host sums the
4 partials per batch in fp32 and adds the (folded) biases.

Numerics: all matmuls run in bf16 with fp32 PSUM accumulation.  Softmax is
computed without max-subtraction (scores are bounded, |s| < ~3.5) with the
normalization deferred to the attention *output*:
    attnT[d, q] = (sum_k v[k, d] * exp(sT[k, q])) / z[q],  z = sum_k exp
The denominator is NOT a full ones-matmul over every e-tile (that would cost
as much tensor time as the pv matmul).  Instead the DVE pre-reduces the 16
key-tiles of e elementwise (a 6-instruction bf16 add-ladder, cheap on the
otherwise idle vector engine), and a SINGLE ones-matmul per chunk
partition-reduces the [128, QC] ladder result, landing z already broadcast
across partitions.  This cuts the z tensor cost 16x (~51us/core).
The v-bias is dropped in-kernel: after normalization it contributes exactly
b_v to every row, so the host folds w_out @ b_v into the output bias.

Layout notes: qT/kT/attnT live as [128 (head-dim), head, seq] so every
matmul contracts over a full 128-partition tile with no transposes anywhere.
The qkv weights share SBUF slots with the attention exp-buffers (tag "e"):
they are dead once the projections finish, exactly when the exp buffers
start rotating.  Attention runs a depth-2 software pipeline (st/exp two
chunks ahead of pv) so the scalar engine's exp throughput (~8.7us/chunk vs
~7.0us of tensor work) never stalls the PE.  Startup DMAs are sliced per
4-ht block, interleaving wq and x so the first q-matmul starts after ~1MB
of traffic instead of 4MB.
"""

import numpy as np
import ml_dtypes

B, S, H = 2, 2048, 2048
NH, HD = 16, 128
P = 128
HT = H // P            # 16 hidden-dim tiles
G = 4                  # heads per core
GH = G * HD            # 512: head-group width per core
SCALE = 1.0 / float(np.sqrt(HD))
N_CORES = 8
XC = 512               # phase-1 x streaming chunk (s elements)
QC = 512               # attention query chunk
KT = S // P            # 16 key tiles

_CACHE = {}


def _build():
    import concourse.mybir as mybir
    import concourse.tile as tile
    from concourse import bacc

    dt = mybir.dt
    Alu = mybir.AluOpType
    Act = mybir.ActivationFunctionType

    nc = bacc.Bacc("TRN2", target_bir_lowering=False, debug=False,
                   enable_asserts=False)

    xt_d = nc.dram_tensor("xt", [H, S], dt.bfloat16, kind="ExternalInput").ap()
    wqt_d = nc.dram_tensor("wqt", [H, GH], dt.bfloat16, kind="ExternalInput").ap()
    wkt_d = nc.dram_tensor("wkt", [H, GH], dt.bfloat16, kind="ExternalInput").ap()
    wvt_d = nc.dram_tensor("wvt", [H, GH], dt.bfloat16, kind="ExternalInput").ap()
    bqs_d = nc.dram_tensor("bqs", [P, G], dt.float32, kind="ExternalInput").ap()
    bk_d = nc.dram_tensor("bk", [P, G], dt.float32, kind="ExternalInput").ap()
    wot_d = nc.dram_tensor("wot", [GH, H], dt.bfloat16, kind="ExternalInput").ap()
    out_d = nc.dram_tensor("partial", [S, H], dt.bfloat16,
                           kind="ExternalOutput").ap()

    xt_r = xt_d.rearrange("(ht p) s -> p ht s", p=P)      # [128, 16, 2048]
    wqt_r = wqt_d.rearrange("(ht p) o -> p ht o", p=P)    # [128, 16, 512]
    wkt_r = wkt_d.rearrange("(ht p) o -> p ht o", p=P)
    wvt_r = wvt_d.rearrange("(ht p) o -> p ht o", p=P)
    wot_r = wot_d.rearrange("(g p) o -> p g o", p=P)      # [128, 4, 2048]

    NXC = S // XC      # 4
    NQC = S // QC      # 4

    with tile.TileContext(nc) as tc:
        with (
            tc.tile_pool(name="consts", bufs=1) as consts,
            tc.tile_pool(name="wpool", bufs=1) as wpool,
            tc.tile_pool(name="xpool", bufs=2) as xpool,
            tc.tile_pool(name="big", bufs=1) as big,
            tc.tile_pool(name="epool", bufs=4) as epool,
            tc.tile_pool(name="small", bufs=2) as small,
            tc.tile_pool(name="psum", bufs=2, space="PSUM") as psum,
        ):
            # --- startup DMAs, critical-path first.  wq slices issue from
            # the Sync queue while the x slices issue in parallel from the
            # (otherwise idle) GpSimd queue: DMA issue instructions cost
            # ~0.7-2us each, so serializing them on one engine delays the
            # first matmul.  First pieces are 2-ht (256KB) so the first
            # q-matmul group (which consumes ht ascending) starts early. ---
            wq_sb = epool.tile([P, HT, GH], dt.bfloat16, tag="e", name="wq_sb")
            xt0_sb = xpool.tile([P, HT, XC], dt.bfloat16, tag="xt",
                                name="xt0_sb")
            for lo, hi in ((0, 2), (2, 4), (4, 8), (8, 12), (12, 16)):
                sl = slice(lo, hi)
                nc.sync.dma_start(wq_sb[:, sl, :], wqt_r[:, sl, :])
                nc.gpsimd.dma_start(xt0_sb[:, sl, :], xt_r[:, sl, 0:XC])
            bqs_sb = consts.tile([P, G], dt.float32)
            nc.gpsimd.dma_start(bqs_sb[:], bqs_d)
            bk_sb = consts.tile([P, G], dt.float32)
            nc.gpsimd.dma_start(bk_sb[:], bk_d)
            ones_sb = consts.tile([P, P], dt.bfloat16)
            nc.vector.memset(ones_sb[:], 1.0)
            # PE warmup: TRN2 ramps the PE clock (0.65 -> 2.4 GHz) only while
            # the array is busy; a few throwaway matmuls during the startup
            # DMA wait mean the first real matmuls run at speed.
            for _ in range(8):
                wps = psum.tile([P, 512], dt.float32, tag="mm")
                nc.tensor.matmul(wps[:, 0:P], ones_sb[:], ones_sb[:],
                                 start=True, stop=True)
            wk_sb = epool.tile([P, HT, GH], dt.bfloat16, tag="e", name="wk_sb")
            nc.sync.dma_start(wk_sb[:, 0:8, :], wkt_r[:, 0:8, :])
            nc.sync.dma_start(wk_sb[:, 8:16, :], wkt_r[:, 8:16, :])
            wv_sb = epool.tile([P, HT, GH], dt.bfloat16, tag="e", name="wv_sb")
            nc.sync.dma_start(wv_sb[:, 0:8, :], wvt_r[:, 0:8, :])
            nc.sync.dma_start(wv_sb[:, 8:16, :], wvt_r[:, 8:16, :])

            qt_sb = big.tile([P, G, S], dt.bfloat16)   # q^T, scale+bias applied
            kt_sb = big.tile([P, G, S], dt.bfloat16)   # k^T, bias applied
            v_sb = big.tile([P, KT, GH], dt.bfloat16)  # v natural [s, o]
            at_sb = big.tile([P, G, S], dt.bfloat16)   # attn output^T

            # ---------------- Phase 1: QKV projections ----------------
            for xc in range(NXC):
                if xc == 0:
                    xt_sb = xt0_sb
                else:
                    xt_sb = xpool.tile([P, HT, XC], dt.bfloat16, tag="xt",
                                       name="xt_sb")
                    nc.sync.dma_start(xt_sb[:], xt_r[:, :, xc * XC:(xc + 1) * XC])
                sl = slice(xc * XC, (xc + 1) * XC)
                for h in range(G):
                    psq = psum.tile([P, 512], dt.float32, tag="mm")
                    for ht in range(HT):
                        nc.tensor.matmul(psq,
                                         wq_sb[:, ht, h * HD:(h + 1) * HD],
                                         xt_sb[:, ht, :],
                                         start=(ht == 0), stop=(ht == HT - 1))
                    nc.vector.tensor_scalar(qt_sb[:, h, sl], psq,
                                            SCALE, bqs_sb[:, h:h + 1],
                                            Alu.mult, Alu.add)
                for h in range(G):
                    psk = psum.tile([P, 512], dt.float32, tag="mm")
                    for ht in range(HT):
                        nc.tensor.matmul(psk,
                                         wk_sb[:, ht, h * HD:(h + 1) * HD],
                                         xt_sb[:, ht, :],
                                         start=(ht == 0), stop=(ht == HT - 1))
                    nc.vector.tensor_scalar_add(kt_sb[:, h, sl], psk,
                                                bk_sb[:, h:h + 1])
                for sv in range(XC // P):
                    sm = xc * (XC // P) + sv
                    psv = psum.tile([P, 512], dt.float32, tag="mm")
                    for ht in range(HT):
                        nc.tensor.matmul(psv,
                                         xt_sb[:, ht, sv * P:(sv + 1) * P],
                                         wv_sb[:, ht, :],
                                         start=(ht == 0), stop=(ht == HT - 1))
                    nc.vector.tensor_copy(out=v_sb[:, sm, :], in_=psv)

            # out-proj weights: needed only from the first proj (~mid-kernel)
            wo_sb = wpool.tile([P, G, H], dt.bfloat16)
            nc.sync.dma_start(wo_sb[:], wot_r)

            # -------- Phase 2+3: attention + out-proj (sw-pipelined) --------
            def emit_st_exp(h, qc):
                # ST^T = k^T.T @ q^T per 128-key tile; exp on ACT in 2-bank
                # batches (halves the per-ACTIVATE overhead).
                e_sb = epool.tile([P, KT, QC], dt.bfloat16, tag="e",
                                  name="e_sb")
                for km in range(0, KT, 2):
                    ps = psum.tile([P, 2, QC], dt.float32, tag="st")
                    for j in range(2):
                        nc.tensor.matmul(ps[:, j, :],
                                         kt_sb[:, h, (km + j) * P:(km + j + 1) * P],
                                         qt_sb[:, h, qc * QC:(qc + 1) * QC],
                                         start=True, stop=True)
                    nc.scalar.activation(e_sb[:, km:km + 2, :], ps, Act.Exp)
                return e_sb

            def emit_pv_z_norm(h, qc, e_sb):
                pv = psum.tile([P, QC], dt.float32, tag="pv", bufs=1)
                for km in range(KT):
                    nc.tensor.matmul(pv, v_sb[:, km, h * HD:(h + 1) * HD],
                                     e_sb[:, km, :],
                                     start=(km == 0), stop=(km == KT - 1))
                # z: DVE add-ladder over the 16 key-tiles of e (bf16, packed
                # SBUF operands -> fast DVE mode), then ONE ones-matmul to
                # partition-reduce, landing z broadcast across partitions.
                lA = small.tile([P, 4, QC], dt.bfloat16, tag="l4")
                nc.vector.tensor_add(out=lA[:], in0=e_sb[:, 0:4, :],
                                     in1=e_sb[:, 4:8, :])
                lB = small.tile([P, 4, QC], dt.bfloat16, tag="l4")
                nc.vector.tensor_add(out=lB[:], in0=e_sb[:, 8:12, :],
                                     in1=e_sb[:, 12:16, :])
                lC = small.tile([P, 2, QC], dt.bfloat16, tag="l2")
                nc.vector.tensor_add(out=lC[:], in0=lA[:, 0:2, :],
                                     in1=lA[:, 2:4, :])
                lD = small.tile([P, 2, QC], dt.bfloat16, tag="l2")
                nc.vector.tensor_add(out=lD[:], in0=lB[:, 0:2, :],
                                     in1=lB[:, 2:4, :])
                lE = small.tile([P, 2, QC], dt.bfloat16, tag="le", bufs=1)
                nc.vector.tensor_add(out=lE[:], in0=lC[:], in1=lD[:])
                esum = small.tile([P, QC], dt.bfloat16, tag="es", bufs=1)
                nc.vector.tensor_add(out=esum[:], in0=lE[:, 0, :],
                                     in1=lE[:, 1, :])
                z = psum.tile([P, QC], dt.float32, tag="z", bufs=1)
                nc.tensor.matmul(z, ones_sb[:], esum[:], start=True, stop=True)
                zi = small.tile([P, QC], dt.float32, tag="zi")
                nc.vector.reciprocal_approx_fast(out=zi[:], in_=z)
                nc.vector.tensor_mul(out=at_sb[:, h, qc * QC:(qc + 1) * QC],
                                     in0=pv, in1=zi[:])

            def emit_proj(qc, last=False):
                for sv in range(QC // P):
                    sm = qc * (QC // P) + sv
                    ob = None
                    for oc in range(H // 512):
                        pp = psum.tile([P, 512], dt.float32, tag="mm")
                        for g in range(G):
                            nc.tensor.matmul(pp,
                                             at_sb[:, g, sm * P:(sm + 1) * P],
                                             wo_sb[:, g, oc * 512:(oc + 1) * 512],
                                             start=(g == 0), stop=(g == G - 1))
                        if oc % 2 == 0:
                            ob = small.tile([P, 2, 512], dt.bfloat16, tag="ob",
                                            bufs=3)
                        # in the final group, split the drain copies across
                        # DVE and ACT so the tail isn't serialized on one
                        # engine (Copy is in every ACT table set: no reload;
                        # GpSimd cannot read PSUM on TRN2).
                        if last and oc % 2 == 1:
                            nc.scalar.copy(ob[:, oc % 2, :], pp)
                        else:
                            nc.vector.tensor_copy(out=ob[:, oc % 2, :], in_=pp)
                        if oc % 2 == 1:
                            nc.sync.dma_start(
                                out_d[sm * P:(sm + 1) * P,
                                      (oc - 1) * 512:(oc + 1) * 512],
                                ob[:])

            chunks = [(h, qc) for qc in range(NQC) for h in range(G)]
            emitted = []
            for i, (h, qc) in enumerate(chunks):
                e = emit_st_exp(h, qc)
                emitted.append((h, qc, e))
                if i >= 2:
                    ph, pqc, pe = emitted[i - 2]
                    emit_pv_z_norm(ph, pqc, pe)
                    if ph == G - 1:
                        emit_proj(pqc)
            for i in (len(chunks) - 2, len(chunks) - 1):
                ph, pqc, pe = emitted[i]
                emit_pv_z_norm(ph, pqc, pe)
                if ph == G - 1:
                    emit_proj(pqc, last=(i == len(chunks) - 1))

    nc.compile()
    return nc


def _get_nc():
    if "nc" not in _CACHE:
        _CACHE["nc"] = _build()
    return _CACHE["nc"]


def _make_in_maps(x, w_qkv, b_qkv, w_out):
    bf = ml_dtypes.bfloat16
    f32 = np.float32
    in_maps = []
    for c in range(N_CORES):
        b = c // 4
        g = c % 4
        lo = GH * g
        hi = GH * (g + 1)
        xt = np.ascontiguousarray(x[b].T).astype(bf)
        wqt = np.ascontiguousarray(w_qkv[lo:hi, :].T).astype(bf)
        wkt = np.ascontiguousarray(w_qkv[H + lo:H + hi, :].T).astype(bf)
        wvt = np.ascontiguousarray(w_qkv[2 * H + lo:2 * H + hi, :].T).astype(bf)
        bqs = np.ascontiguousarray(
            (b_qkv[lo:hi] * SCALE).astype(f32).reshape(G, P).T)
        bk = np.ascontiguousarray(
            b_qkv[H + lo:H + hi].astype(f32).reshape(G, P).T)
        wot = np.ascontiguousarray(w_out[:, lo:hi].T).astype(bf)
        in_maps.append({"xt": xt, "wqt": wqt, "wkt": wkt, "wvt": wvt,
                        "bqs": bqs, "bk": bk, "wot": wot})
    return in_maps


def kernel(x, w_qkv, b_qkv, w_out, b_out):
    import os
    import sys

    x = np.asarray(x, dtype=np.float32)
    w_qkv = np.asarray(w_qkv, dtype=np.float32)
    b_qkv = np.asarray(b_qkv, dtype=np.float32)
    w_out = np.asarray(w_out, dtype=np.float32)
    b_out = np.asarray(b_out, dtype=np.float32)

    from concourse.bass_utils import run_bass_kernel_spmd

    # NTFF tracing under axon needs the antenv.axon_hooks shim (test.py
    # installs it); without it a stray BASS_TRACE=1 in the environment would
    # crash the run — disable tracing in that case.
    if "antenv.axon_hooks" not in sys.modules:
        os.environ["BASS_NEVER_TRACE"] = "1"

    nc = _get_nc()
    in_maps = _make_in_maps(x, w_qkv, b_qkv, w_out)
    res = run_bass_kernel_spmd(nc, in_maps, core_ids=list(range(N_CORES)))
    _CACHE["last_results"] = res
    partials = [r["partial"] for r in res.results]

    bv = b_qkv[2 * H:3 * H]
    bias = b_out + w_out @ bv          # folded v-bias contribution
    out = np.empty((B, S, H), np.float32)
    for b in range(B):
        acc = partials[4 * b].astype(np.float32)
        for g in range(1, 4):
            acc += partials[4 * b + g].astype(np.float32)
        out[b] = acc + bias
    return out


# revision 11
# speedup vs baseline: 1.1706x; 1.1706x over previous
"""Trainium2 Bass kernel for NoTPAttention (dense transformer block:
fused QKV projection -> multi-head attention -> output projection).

Sharding (8 NeuronCores): core c handles batch b = c // 4 and the 4 heads
g = 4*(c % 4) .. 4*(c % 4)+3 (head-parallel tensor parallelism).  Each core
computes its heads' partial out-projection [S, H] in bf16; the host sums the
4 partials per batch in fp32 and adds the (folded) biases.

Numerics: all matmuls run in bf16 with fp32 PSUM accumulation.  Softmax is
computed without max-subtraction (scores are bounded, |s| < ~3.5) with the
normalization deferred to the attention *output*:
    attnT[d, q] = (sum_k v[k, d] * exp(sT[k, q])) / z[q],  z = sum_k exp
The denominator is NOT a full ones-matmul over every e-tile (that would cost
as much tensor time as the pv matmul).  Instead the DVE pre-reduces the 16
key-tiles of e elementwise (a 6-instruction bf16 add-ladder, cheap on the
otherwise idle vector engine), and a SINGLE ones-matmul per chunk
partition-reduces the [128, QC] ladder result, landing z already broadcast
across partitions.  This cuts the z tensor cost 16x (~51us/core).
The v-bias is dropped in-kernel: after normalization it contributes exactly
b_v to every row, so the host folds w_out @ b_v into the output bias.

Layout notes: qT/kT/attnT live as [128 (head-dim), head, seq] so every
matmul contracts over a full 128-partition tile with no transposes anywhere.
The qkv weights share SBUF slots with the attention exp-buffers (tag "e"):
they are dead once the projections finish, exactly when the exp buffers
start rotating.  Attention runs a depth-2 software pipeline (st/exp two
chunks ahead of pv) so the scalar engine's exp throughput (~8.7us/chunk vs
~7.0us of tensor work) never stalls the PE.  Startup DMAs are sliced per
4-ht block, interleaving wq and x so the first q-matmul starts after ~1MB
of traffic instead of 4MB.
"""

import numpy as np
import ml_dtypes

B, S, H = 2, 2048, 2048
NH, HD = 16, 128
P = 128
HT = H // P            # 16 hidden-dim tiles
G = 4                  # heads per core
GH = G * HD            # 512: head-group width per core
SCALE = 1.0 / float(np.sqrt(HD))
N_CORES = 8
XC = 512               # phase-1 x streaming chunk (s elements)
QC = 512               # attention query chunk
KT = S // P            # 16 key tiles

_CACHE = {}


def _build():
    import concourse.mybir as mybir
    import concourse.tile as tile
    from concourse import bacc

    dt = mybir.dt
    Alu = mybir.AluOpType
    Act = mybir.ActivationFunctionType

    nc = bacc.Bacc("TRN2", target_bir_lowering=False, debug=False,
                   enable_asserts=False)

    xt_d = nc.dram_tensor("xt", [H, S], dt.bfloat16, kind="ExternalInput").ap()
    # wq is head-major ([g, H, hd] flattened) so each head's 0.5MB block is
    # contiguous in DRAM: the first q-matmul group only waits on head 0's
    # block instead of the full 2MB weight.
    wqt_d = nc.dram_tensor("wqt", [G * H, HD], dt.bfloat16,
                           kind="ExternalInput").ap()
    wkt_d = nc.dram_tensor("wkt", [H, GH], dt.bfloat16, kind="ExternalInput").ap()
    wvt_d = nc.dram_tensor("wvt", [H, GH], dt.bfloat16, kind="ExternalInput").ap()
    bqs_d = nc.dram_tensor("bqs", [P, G], dt.float32, kind="ExternalInput").ap()
    bk_d = nc.dram_tensor("bk", [P, G], dt.float32, kind="ExternalInput").ap()
    wot_d = nc.dram_tensor("wot", [GH, H], dt.bfloat16, kind="ExternalInput").ap()
    out_d = nc.dram_tensor("partial", [S, H], dt.bfloat16,
                           kind="ExternalOutput").ap()

    xt_r = xt_d.rearrange("(ht p) s -> p ht s", p=P)      # [128, 16, 2048]
    wqt_r = wqt_d.rearrange("(g ht p) d -> p g ht d", g=G, p=P)  # [128,4,16,128]
    wkt_r = wkt_d.rearrange("(ht p) o -> p ht o", p=P)
    wvt_r = wvt_d.rearrange("(ht p) o -> p ht o", p=P)
    wot_r = wot_d.rearrange("(g p) o -> p g o", p=P)      # [128, 4, 2048]

    NXC = S // XC      # 4
    NQC = S // QC      # 4

    with tile.TileContext(nc) as tc:
        with (
            tc.tile_pool(name="consts", bufs=1) as consts,
            tc.tile_pool(name="wpool", bufs=1) as wpool,
            tc.tile_pool(name="xpool", bufs=2) as xpool,
            tc.tile_pool(name="big", bufs=1) as big,
            tc.tile_pool(name="epool", bufs=4) as epool,
            tc.tile_pool(name="small", bufs=2) as small,
            tc.tile_pool(name="psum", bufs=2, space="PSUM") as psum,
        ):
            # --- startup DMAs, critical-path first.  wq slices issue from
            # the Sync queue while the x slices issue in parallel from the
            # (otherwise idle) GpSimd queue: DMA issue instructions cost
            # ~0.7-2us each, so serializing them on one engine delays the
            # first matmul.  First pieces are 2-ht (256KB) so the first
            # q-matmul group (which consumes ht ascending) starts early. ---
            wq_sb = epool.tile([P, HT, GH], dt.bfloat16, tag="e", name="wq_sb")
            xt0_sb = xpool.tile([P, HT, XC], dt.bfloat16, tag="xt",
                                name="xt0_sb")
            for h in range(G):
                nc.sync.dma_start(wq_sb[:, :, h * HD:(h + 1) * HD],
                                  wqt_r[:, h, :, :])
            for lo, hi in ((0, 2), (2, 4), (4, 8), (8, 12), (12, 16)):
                sl = slice(lo, hi)
                nc.gpsimd.dma_start(xt0_sb[:, sl, :], xt_r[:, sl, 0:XC])
            bqs_sb = consts.tile([P, G], dt.float32)
            nc.gpsimd.dma_start(bqs_sb[:], bqs_d)
            bk_sb = consts.tile([P, G], dt.float32)
            nc.gpsimd.dma_start(bk_sb[:], bk_d)
            ones_sb = consts.tile([P, P], dt.bfloat16)
            nc.vector.memset(ones_sb[:], 1.0)
            warm_sb = consts.tile([P, 512], dt.bfloat16)
            nc.vector.memset(warm_sb[:], 0.5)
            # PE warmup: TRN2 ramps the PE clock (0.65 -> 2.4 GHz) only while
            # the array is busy; throwaway matmuls during the startup DMA
            # wait mean the first real matmuls run at speed.
            for _ in range(8):
                wps = psum.tile([P, 512], dt.float32, tag="mm")
                nc.tensor.matmul(wps[:], ones_sb[:], warm_sb[:],
                                 start=True, stop=True)
            wk_sb = epool.tile([P, HT, GH], dt.bfloat16, tag="e", name="wk_sb")
            nc.sync.dma_start(wk_sb[:, 0:8, :], wkt_r[:, 0:8, :])
            nc.sync.dma_start(wk_sb[:, 8:16, :], wkt_r[:, 8:16, :])
            wv_sb = epool.tile([P, HT, GH], dt.bfloat16, tag="e", name="wv_sb")
            nc.sync.dma_start(wv_sb[:, 0:8, :], wvt_r[:, 0:8, :])
            nc.sync.dma_start(wv_sb[:, 8:16, :], wvt_r[:, 8:16, :])

            qt_sb = big.tile([P, G, S], dt.bfloat16)   # q^T, scale+bias applied
            kt_sb = big.tile([P, G, S], dt.bfloat16)   # k^T, bias applied
            v_sb = big.tile([P, KT, GH], dt.bfloat16)  # v natural [s, o]
            at_sb = big.tile([P, G, S], dt.bfloat16)   # attn output^T

            # ---------------- Phase 1: QKV projections ----------------
            for xc in range(NXC):
                if xc == 0:
                    xt_sb = xt0_sb
                else:
                    xt_sb = xpool.tile([P, HT, XC], dt.bfloat16, tag="xt",
                                       name="xt_sb")
                    nc.sync.dma_start(xt_sb[:], xt_r[:, :, xc * XC:(xc + 1) * XC])
                sl = slice(xc * XC, (xc + 1) * XC)
                for h in range(G):
                    psq = psum.tile([P, 512], dt.float32, tag="mm")
                    for ht in range(HT):
                        nc.tensor.matmul(psq,
                                         wq_sb[:, ht, h * HD:(h + 1) * HD],
                                         xt_sb[:, ht, :],
                                         start=(ht == 0), stop=(ht == HT - 1))
                    nc.vector.tensor_scalar(qt_sb[:, h, sl], psq,
                                            SCALE, bqs_sb[:, h:h + 1],
                                            Alu.mult, Alu.add)
                for h in range(G):
                    psk = psum.tile([P, 512], dt.float32, tag="mm")
                    for ht in range(HT):
                        nc.tensor.matmul(psk,
                                         wk_sb[:, ht, h * HD:(h + 1) * HD],
                                         xt_sb[:, ht, :],
                                         start=(ht == 0), stop=(ht == HT - 1))
                    nc.vector.tensor_scalar_add(kt_sb[:, h, sl], psk,
                                                bk_sb[:, h:h + 1])
                for sv in range(XC // P):
                    sm = xc * (XC // P) + sv
                    psv = psum.tile([P, 512], dt.float32, tag="mm")
                    for ht in range(HT):
                        nc.tensor.matmul(psv,
                                         xt_sb[:, ht, sv * P:(sv + 1) * P],
                                         wv_sb[:, ht, :],
                                         start=(ht == 0), stop=(ht == HT - 1))
                    nc.vector.tensor_copy(out=v_sb[:, sm, :], in_=psv)

            # out-proj weights: needed only from the first proj (~mid-kernel)
            wo_sb = wpool.tile([P, G, H], dt.bfloat16)
            nc.sync.dma_start(wo_sb[:], wot_r)

            # -------- Phase 2+3: attention + out-proj (sw-pipelined) --------
            def emit_st_exp(h, qc):
                # ST^T = k^T.T @ q^T per 128-key tile; exp on ACT in 2-bank
                # batches (halves the per-ACTIVATE overhead).
                e_sb = epool.tile([P, KT, QC], dt.bfloat16, tag="e",
                                  name="e_sb")
                for km in range(0, KT, 2):
                    ps = psum.tile([P, 2, QC], dt.float32, tag="st")
                    for j in range(2):
                        nc.tensor.matmul(ps[:, j, :],
                                         kt_sb[:, h, (km + j) * P:(km + j + 1) * P],
                                         qt_sb[:, h, qc * QC:(qc + 1) * QC],
                                         start=True, stop=True)
                    nc.scalar.activation(e_sb[:, km:km + 2, :], ps, Act.Exp)
                return e_sb

            def emit_pv_z_norm(h, qc, e_sb):
                pv = psum.tile([P, QC], dt.float32, tag="pv", bufs=2)
                for km in range(KT):
                    nc.tensor.matmul(pv, v_sb[:, km, h * HD:(h + 1) * HD],
                                     e_sb[:, km, :],
                                     start=(km == 0), stop=(km == KT - 1))
                # z: DVE add-ladder over the 16 key-tiles of e (bf16, packed
                # SBUF operands -> fast DVE mode), then ONE ones-matmul to
                # partition-reduce, landing z broadcast across partitions.
                lA = small.tile([P, 4, QC], dt.bfloat16, tag="l4")
                nc.vector.tensor_add(out=lA[:], in0=e_sb[:, 0:4, :],
                                     in1=e_sb[:, 4:8, :])
                lB = small.tile([P, 4, QC], dt.bfloat16, tag="l4")
                nc.vector.tensor_add(out=lB[:], in0=e_sb[:, 8:12, :],
                                     in1=e_sb[:, 12:16, :])
                lC = small.tile([P, 2, QC], dt.bfloat16, tag="l2")
                nc.vector.tensor_add(out=lC[:], in0=lA[:, 0:2, :],
                                     in1=lA[:, 2:4, :])
                lD = small.tile([P, 2, QC], dt.bfloat16, tag="l2")
                nc.vector.tensor_add(out=lD[:], in0=lB[:, 0:2, :],
                                     in1=lB[:, 2:4, :])
                lE = small.tile([P, 2, QC], dt.bfloat16, tag="le", bufs=1)
                nc.vector.tensor_add(out=lE[:], in0=lC[:], in1=lD[:])
                esum = small.tile([P, QC], dt.bfloat16, tag="es", bufs=1)
                nc.vector.tensor_add(out=esum[:], in0=lE[:, 0, :],
                                     in1=lE[:, 1, :])
                # z shares the "mm" psum ring (its own bank would push the
                # total over 8 now that pv is double-buffered)
                z = psum.tile([P, QC], dt.float32, tag="mm")
                nc.tensor.matmul(z, ones_sb[:], esum[:], start=True, stop=True)
                zi = small.tile([P, QC], dt.float32, tag="zi")
                nc.vector.reciprocal_approx_fast(out=zi[:], in_=z)
                nc.vector.tensor_mul(out=at_sb[:, h, qc * QC:(qc + 1) * QC],
                                     in0=pv, in1=zi[:])

            def emit_proj(qc, last=False):
                for sv in range(QC // P):
                    sm = qc * (QC // P) + sv
                    ob = None
                    for oc in range(H // 512):
                        pp = psum.tile([P, 512], dt.float32, tag="mm")
                        for g in range(G):
                            nc.tensor.matmul(pp,
                                             at_sb[:, g, sm * P:(sm + 1) * P],
                                             wo_sb[:, g, oc * 512:(oc + 1) * 512],
                                             start=(g == 0), stop=(g == G - 1))
                        if oc % 2 == 0:
                            ob = small.tile([P, 2, 512], dt.bfloat16, tag="ob",
                                            bufs=3)
                        # in the final group, split the drain copies across
                        # DVE and ACT so the tail isn't serialized on one
                        # engine (Copy is in every ACT table set: no reload;
                        # GpSimd cannot read PSUM on TRN2).
                        if last and oc % 2 == 1:
                            nc.scalar.copy(ob[:, oc % 2, :], pp)
                        else:
                            nc.vector.tensor_copy(out=ob[:, oc % 2, :], in_=pp)
                        if oc % 2 == 1:
                            nc.sync.dma_start(
                                out_d[sm * P:(sm + 1) * P,
                                      (oc - 1) * 512:(oc + 1) * 512],
                                ob[:])

            chunks = [(h, qc) for qc in range(NQC) for h in range(G)]
            emitted = []
            for i, (h, qc) in enumerate(chunks):
                e = emit_st_exp(h, qc)
                emitted.append((h, qc, e))
                if i >= 2:
                    ph, pqc, pe = emitted[i - 2]
                    emit_pv_z_norm(ph, pqc, pe)
                    if ph == G - 1:
                        emit_proj(pqc)
            for i in (len(chunks) - 2, len(chunks) - 1):
                ph, pqc, pe = emitted[i]
                emit_pv_z_norm(ph, pqc, pe)
                if ph == G - 1:
                    emit_proj(pqc, last=(i == len(chunks) - 1))

    nc.compile()
    return nc


def _get_nc():
    if "nc" not in _CACHE:
        _CACHE["nc"] = _build()
    return _CACHE["nc"]


def _make_in_maps(x, w_qkv, b_qkv, w_out):
    bf = ml_dtypes.bfloat16
    f32 = np.float32
    in_maps = []
    for c in range(N_CORES):
        b = c // 4
        g = c % 4
        lo = GH * g
        hi = GH * (g + 1)
        xt = np.ascontiguousarray(x[b].T).astype(bf)
        # head-major wq: [g, H, hd] flattened to [g*H, hd]
        wqt = np.ascontiguousarray(
            w_qkv[lo:hi, :].T.reshape(H, G, HD).transpose(1, 0, 2)
            .reshape(G * H, HD)).astype(bf)
        wkt = np.ascontiguousarray(w_qkv[H + lo:H + hi, :].T).astype(bf)
        wvt = np.ascontiguousarray(w_qkv[2 * H + lo:2 * H + hi, :].T).astype(bf)
        bqs = np.ascontiguousarray(
            (b_qkv[lo:hi] * SCALE).astype(f32).reshape(G, P).T)
        bk = np.ascontiguousarray(
            b_qkv[H + lo:H + hi].astype(f32).reshape(G, P).T)
        wot = np.ascontiguousarray(w_out[:, lo:hi].T).astype(bf)
        in_maps.append({"xt": xt, "wqt": wqt, "wkt": wkt, "wvt": wvt,
                        "bqs": bqs, "bk": bk, "wot": wot})
    return in_maps


def kernel(x, w_qkv, b_qkv, w_out, b_out):
    import os
    import sys

    x = np.asarray(x, dtype=np.float32)
    w_qkv = np.asarray(w_qkv, dtype=np.float32)
    b_qkv = np.asarray(b_qkv, dtype=np.float32)
    w_out = np.asarray(w_out, dtype=np.float32)
    b_out = np.asarray(b_out, dtype=np.float32)

    from concourse.bass_utils import run_bass_kernel_spmd

    # NTFF tracing under axon needs the antenv.axon_hooks shim (test.py
    # installs it); without it a stray BASS_TRACE=1 in the environment would
    # crash the run — disable tracing in that case.
    if "antenv.axon_hooks" not in sys.modules:
        os.environ["BASS_NEVER_TRACE"] = "1"

    nc = _get_nc()
    in_maps = _make_in_maps(x, w_qkv, b_qkv, w_out)
    res = run_bass_kernel_spmd(nc, in_maps, core_ids=list(range(N_CORES)))
    _CACHE["last_results"] = res
    partials = [r["partial"] for r in res.results]

    bv = b_qkv[2 * H:3 * H]
    bias = b_out + w_out @ bv          # folded v-bias contribution
    out = np.empty((B, S, H), np.float32)
    for b in range(B):
        acc = partials[4 * b].astype(np.float32)
        for g in range(1, 4):
            acc += partials[4 * b + g].astype(np.float32)
        out[b] = acc + bias
    return out
